# revision 35
# baseline (speedup 1.0000x reference)
"""MHA forward kernel for Trainium2 (Bass/Tile), sharded over (batch, head)
pairs across 8 NeuronCores.

Math (per (b,h) pair):
    scores = softmax(Q K^T / sqrt(64) + bias)   # bias broadcast over (b,h)
    out    = scores @ V

Device-side layout: computed TRANSPOSED (scoresT[k, q]) so the S x S scores
matrix never needs an on-chip transpose. The softmax normalization and the
final [d, q] -> [q, d] transpose happen HOST-side: the kernel emits, per
q-tile, outT[d, q] (rows 0..63) and sums[q] (row 64, via a ones-column
appended to V), unnormalized.

Engine plan (the exp of S^2 elements is the fundamental bottleneck):
  - PE: every matmul runs in 64-row tiling mode (tile_size (64,128)) so the
    engine never pays a tiling-mode drain:
      MM1: scoresT k-tile pairs as dual row-tiles (T0 = partitions 0-63,
           T8 = 64-127; Q,K duplicated into both partition halves host-side)
           -> 2 k-tiles per 512-cycle stream.
      MM2: each k-tile's 128-deep contraction split into two 64-row halves
           accumulating into separate PSUM banks o_a / o_b (concurrent
           duals); o = o_a + o_b folded into the qt drain.
  - bias: exp(s+b) = exp(s)*exp(b). Per k-tile group one of:
      "mult":  ACT exp (PSUM f32 -> SBUF bf16), then multiply by
               host-precomputed exp(biasT) (bf16) on DVE or GPSIMD.
      "sch":   DVE-only 2^x via int32 Schraudolph: one scalar_tensor_tensor
               i32 = rint(s*C + bprep[k,q]), C = 2^23*log2e, bprep folds the
               bias, the 127-exponent offset and a mean-centering shift;
               MM2 reads the int32 tile bitcast to float32r (1 cyc/row at
               N=512). Ripple of (1+f)/2^f costs ~6e-3 rel err at 5/16
               of k-tiles (measured vs reference: ~1.05e-2 total).
"""

import os
import sys

import numpy as np

for _p in ("/opt/trn_rl_repo",):
    if _p not in sys.path and os.path.isdir(_p):
        sys.path.insert(0, _p)

B, H, S, D = 2, 16, 2048, 64
N_CORES = 8
PAIRS = B * H                     # 32
PPC = PAIRS // N_CORES            # 4 pairs per core
SCALE = 1.0 / 8.0                 # 1/sqrt(64)

KT = S // 128                     # 16 k-tiles of 128
QTILE = 512
QT = S // QTILE                   # 4 q-tiles

LOG2E = 1.4426950408889634
# int16 Schraudolph: i16 = rint(s*C16 + bprep) bit-patterns a bf16 (1-8-7).
# The precompute tensor is f32 (DVE sums in f32 internally), so the large
# 127*128 exponent offset costs no precision; only the final int16 convert
# quantizes (2^-8 relative, same as the bf16 probs of the mult path).
C16 = float(np.float32(LOG2E * 128.0))
# exponent offset with mean-centering shift (E[(1+f)/2^f] = 1.0418)
SCH_OFF = float(np.float32((127.0 - 0.05915) * 128.0))

# per-qt k-tile groups: (kt0, n_kt, path). Paths: "mult_dve" / "mult_gps"
# (ACT exp + multiply on that engine) or "sch" (DVE Schraudolph).
# The two gps groups are spread apart (their 2.6-3us multiplies queue
# behind each other on the GPSIMD engine otherwise), and each path has its
# own consume lag (below): gps chains are ~3us longer than dve/sch chains.
GROUPS = [
    (0, 3, "mult_gps"),
    (3, 3, "mult_dve"),
    (6, 3, "sch"),
    (9, 3, "sch"),
    (12, 2, "mult_gps"),
    (14, 2, "mult_dve"),
]
_env_paths = os.environ.get("KPATHS")
if _env_paths:
    _p = _env_paths.split(",")
    GROUPS = [(g[0], g[1], _p[i]) for i, g in enumerate(GROUPS)]

MULT_KTS = [g[0] + j for g in GROUPS if g[2].startswith("mult") for j in range(g[1])]
SCH_KTS = [g[0] + j for g in GROUPS if g[2] == "sch" for j in range(g[1])]
N_MULT = len(MULT_KTS)
N_SCH = len(SCH_KTS)
# group -> slot offset within its bias-class tensor
_moff, _soff, MULT_OFF, SCH_OFF_IDX = 0, 0, {}, {}
for gi, (kt0, n, path) in enumerate(GROUPS):
    if path == "sch":
        SCH_OFF_IDX[gi] = _soff
        _soff += n
    else:
        MULT_OFF[gi] = _moff
        _moff += n

LAG = int(os.environ.get("LAG", "3"))
LAG_GPS = int(os.environ.get("LAG_GPS", "6"))
PP_BUFS = int(os.environ.get("PP_BUFS", "5"))
P32_BUFS = int(os.environ.get("P32_BUFS", "3"))
SC_BUFS = int(os.environ.get("SC_BUFS", "2"))

_CACHE = {}


def _build_nc():
    import concourse.mybir as mybir
    import concourse.tile as tile
    from concourse import bacc

    f32 = mybir.dt.float32
    bf16 = mybir.dt.bfloat16
    i16 = mybir.dt.int16
    nc = bacc.Bacc(None)

    qT2 = nc.declare_dram_parameter("qT2", [PPC, 128, S], bf16, isOutput=False)
    kT2 = nc.declare_dram_parameter("kT2", [PPC, 128, S], bf16, isOutput=False)
    v1 = nc.declare_dram_parameter("v1", [PPC, 128, KT, D + 1], bf16, isOutput=False)
    ebT_d = nc.declare_dram_parameter("ebT", [128, max(N_MULT, 1), S], bf16, isOutput=False)
    bp_d = nc.declare_dram_parameter("bp32", [128, max(N_SCH, 1), S], f32, isOutput=False)
    out = nc.declare_dram_parameter("out", [PPC, 2, D + 1, S], f32, isOutput=True)

    mult = mybir.AluOpType.mult
    add = mybir.AluOpType.add
    Exp = mybir.ActivationFunctionType.Exp

    with tile.TileContext(nc) as tc:
        with (
            tc.tile_pool(name="bias", bufs=1) as bias_pool,
            tc.tile_pool(name="qk", bufs=2) as qk_pool,
            tc.tile_pool(name="vv", bufs=2) as v_pool,
            tc.tile_pool(name="probP", bufs=PP_BUFS) as pP_pool,
            tc.tile_pool(name="prob16", bufs=P32_BUFS) as p16_pool,
            tc.tile_pool(name="osb", bufs=2) as osb_pool,
            tc.tile_pool(name="sc", bufs=SC_BUFS, space="PSUM") as sc_pool,
            tc.tile_pool(name="oab", bufs=1, space="PSUM") as oab_pool,
        ):
            def load_pair(p):
                q_sb = qk_pool.tile([128, S], bf16, tag="q")
                nc.sync.dma_start(q_sb[:], qT2[p])
                k_sb = qk_pool.tile([128, S], bf16, tag="k")
                nc.sync.dma_start(k_sb[:], kT2[p])
                v_sb = v_pool.tile([128, KT, D + 1], bf16)
                nc.sync.dma_start(v_sb[:], v1[p])
                return q_sb, k_sb, v_sb

            loaded = {0: load_pair(0)}

            # bias-derived tensors resident in SBUF; DMA'd q-column-major so
            # the qt=0 slices land first.
            ebT_sb = bias_pool.tile([128, max(N_MULT, 1), S], bf16)
            bp_sb = bias_pool.tile([128, max(N_SCH, 1), S], f32)
            for qc in range(QT):
                qsl = slice(qc * QTILE, (qc + 1) * QTILE)
                if N_MULT:
                    nc.sync.dma_start(ebT_sb[:, :N_MULT, qsl], ebT_d[:, :N_MULT, qsl])
                if N_SCH:
                    nc.sync.dma_start(bp_sb[:, :N_SCH, qsl], bp_d[:, :N_SCH, qsl])

            def produce(p, qt, gi):
                q_sb, k_sb, _ = loaded[p]
                kt0, n, path = GROUPS[gi]
                qsl = slice(qt * QTILE, (qt + 1) * QTILE)
                s_ps = sc_pool.tile([128, 3, QTILE], f32, tag="sc")
                # MM1: dual row-tiled pairs (T0 = partitions 0-63 computes
                # k-tile kt0+2j, T8 = 64-127 computes kt0+2j+1), then a
                # lone T0 matmul for an odd trailing k-tile.
                for j in range(0, n - 1, 2):
                    ka, kb = kt0 + j, kt0 + j + 1
                    nc.tensor.matmul(
                        s_ps[:, j, :],
                        k_sb[0:64, ka * 128 : (ka + 1) * 128],
                        q_sb[0:64, qsl],
                        start=True, stop=True,
                    )
                    nc.tensor.matmul(
                        s_ps[:, j + 1, :],
                        k_sb[64:128, kb * 128 : (kb + 1) * 128],
                        q_sb[64:128, qsl],
                        start=True, stop=True,
                    )
                if n % 2:
                    kc = kt0 + n - 1
                    nc.tensor.matmul(
                        s_ps[:, n - 1, :],
                        k_sb[0:64, kc * 128 : (kc + 1) * 128],
                        q_sb[0:64, qsl],
                        start=True, stop=True,
                    )
                if path == "sch":
                    soff = SCH_OFF_IDX[gi]
                    p16 = p16_pool.tile([128, 3, QTILE], i16, tag="p16")
                    nc.vector.scalar_tensor_tensor(
                        p16[:, :n, :],
                        s_ps[:, :n, :],
                        C16,
                        bp_sb[:, soff : soff + n, qsl],
                        op0=mult,
                        op1=add,
                    )
                    return (kt0, n, p16, True)
                moff = MULT_OFF[gi]
                p_sb = pP_pool.tile([128, 3, QTILE], bf16, tag="pP")
                nc.scalar.activation(p_sb[:, :n, :], s_ps[:, :n, :], Exp)
                eng = nc.gpsimd if path == "mult_gps" else nc.vector
                eng.tensor_tensor(
                    p_sb[:, :n, :],
                    p_sb[:, :n, :],
                    ebT_sb[:, moff : moff + n, qsl],
                    mult,
                )
                return (kt0, n, p_sb, False)

            def consume(p, qt, chunk, st):
                _, _, v_sb = loaded[p]
                kt0, n, p_t, is16 = chunk
                o_a, o_b = st["o_a"], st["o_b"]
                base = st["consumed"]
                for j in range(n):
                    kt = kt0 + j
                    rhs = p_t[:, j, :]
                    if is16:
                        rhs = rhs.bitcast(bf16)
                    # groups are consumed out of kt order (per-path lags), so
                    # the PSUM accumulation flags key on consume position
                    start = base + j == 0
                    stop = base + j == KT - 1
                    nc.tensor.matmul(
                        o_a[:],
                        v_sb[0:64, kt, :],
                        rhs[0:64, :],
                        start=start, stop=stop,
                    )
                    nc.tensor.matmul(
                        o_b[:],
                        v_sb[64:128, kt, :],
                        rhs[64:128, :],
                        start=start, stop=stop,
                    )
                st["consumed"] += n

            def epilogue(p, qt, st):
                # Drain o_a via ACT and o_b via DVE as two INDEPENDENT
                # copies (DVE can read only one PSUM operand per op, and a
                # staged add would chain the two engines); the host adds the
                # halves together with the softmax normalization.
                qsl = slice(qt * QTILE, (qt + 1) * QTILE)
                t_sb = osb_pool.tile([D + 1, QTILE], f32, tag="ta")
                nc.scalar.copy(t_sb[:], st["o_a"][:])
                nc.sync.dma_start(out[p, 0, :, qsl], t_sb[:])
                o_sb = osb_pool.tile([D + 1, QTILE], f32, tag="osb")
                nc.vector.tensor_scalar_mul(o_sb[:], st["o_b"][:], 1.0)
                nc.sync.dma_start(out[p, 1, :, qsl], o_sb[:])

            stream = []
            for p in range(PPC):
                for qt in range(QT):
                    for gi in range(len(GROUPS)):
                        stream.append((p, qt, gi))

            state = {}
            pending = []  # (due_step, seq, p, qt, chunk)

            def drain_due(step):
                due = sorted(c for c in pending if c[0] <= step)
                for c in due:
                    pending.remove(c)
                    _, _, pp, pq, pc = c
                    st = state[(pp, pq)]
                    consume(pp, pq, pc, st)
                    if st["consumed"] == KT:
                        epilogue(pp, pq, state.pop((pp, pq)))

            for step, (p, qt, gi) in enumerate(stream):
                if p not in loaded:
                    loaded[p] = load_pair(p)
                if qt == QT - 1 and p + 1 < PPC and p + 1 not in loaded:
                    loaded[p + 1] = load_pair(p + 1)
                for old in [k for k in loaded if k < p - 1]:
                    del loaded[old]
                if (p, qt) not in state:
                    o_a = oab_pool.tile([D + 1, QTILE], f32, name="oa", tag="oa")
                    o_b = oab_pool.tile([D + 1, QTILE], f32, name="ob", tag="ob")
                    state[(p, qt)] = {"o_a": o_a, "o_b": o_b, "consumed": 0}
                # consume due chunks BEFORE producing: their probs are older,
                # so the PE queue head is less likely to stall on a wait.
                drain_due(step)
                chunk = produce(p, qt, gi)
                lag = LAG_GPS if GROUPS[gi][2] == "mult_gps" else LAG
                pending.append((step + lag, step, p, qt, chunk))
            step = len(stream)
            while pending:
                drain_due(step)
                step += 1

    return nc


def _get_nc():
    if "nc" not in _CACHE:
        nc = _build_nc()
        nc.finalize()
        _CACHE["nc"] = nc
    return _CACHE["nc"]


def _make_in_maps(mat1, mat2, mat3, bias):
    import ml_dtypes

    bf16 = ml_dtypes.bfloat16
    q = (np.asarray(mat1, dtype=np.float32).reshape(PAIRS, S, D) * SCALE)
    k = np.asarray(mat2, dtype=np.float32).reshape(PAIRS, S, D)
    v = np.asarray(mat3, dtype=np.float32).reshape(PAIRS, S, D)

    qT = q.transpose(0, 2, 1).astype(bf16)          # [P, 64, S]
    kT = k.transpose(0, 2, 1).astype(bf16)
    qT2 = np.concatenate([qT, qT], axis=1)          # [P, 128, S]
    kT2 = np.concatenate([kT, kT], axis=1)
    qT2 = np.ascontiguousarray(qT2)
    kT2 = np.ascontiguousarray(kT2)

    v1f = np.concatenate([v, np.ones((PAIRS, S, 1), np.float32)], axis=2)
    # [P, S, 65] -> [P, 128, KT, 65]: partition = k % 128, slot = k // 128
    v1t = v1f.reshape(PAIRS, KT, 128, D + 1).transpose(0, 2, 1, 3)
    v1 = np.ascontiguousarray(v1t.astype(bf16))

    bT = np.asarray(bias, dtype=np.float32).reshape(S, S).T  # [k, q]
    # [k, q] -> [128, KT, S] (partition = k % 128)
    bT_t = np.ascontiguousarray(bT.reshape(KT, 128, S).transpose(1, 0, 2))
    if N_MULT:
        ebT = np.ascontiguousarray(np.exp(bT_t[:, MULT_KTS, :]).astype(bf16))
    else:
        ebT = np.zeros((128, 1, S), bf16)
    if N_SCH:
        bp32 = np.ascontiguousarray(
            (bT_t[:, SCH_KTS, :] * np.float32(C16) + np.float32(SCH_OFF)).astype(
                np.float32
            )
        )
    else:
        bp32 = np.zeros((128, 1, S), np.float32)

    in_maps = []
    for c in range(N_CORES):
        sl = slice(c * PPC, (c + 1) * PPC)
        in_maps.append(
            {
                "qT2": qT2[sl],
                "kT2": kT2[sl],
                "v1": v1[sl],
                "ebT": ebT,
                "bp32": bp32,
            }
        )
    return in_maps


def kernel(mat1, mat2, mat3, bias):
    from concourse.bass_utils import run_bass_kernel_spmd

    in_maps = _make_in_maps(mat1, mat2, mat3, bias)
    nc = _get_nc()
    _CACHE["in_maps"] = in_maps
    res = run_bass_kernel_spmd(nc, in_maps, list(range(N_CORES)))
    outs = [res.results[c]["out"] for c in range(N_CORES)]
    o2 = np.concatenate(outs, axis=0)                # [PAIRS, 2, 65, S] f32
    o = o2[:, 0] + o2[:, 1]                          # [PAIRS, 65, S]
    full = (o[:, :D, :] / o[:, D : D + 1, :]).transpose(0, 2, 1)
    return np.ascontiguousarray(full.reshape(B, H, S, D).astype(np.float32))


# revision 40
# speedup vs baseline: 1.0719x; 1.0719x over previous
"""MHA forward kernel for Trainium2 (Bass/Tile), sharded over (batch, head)
pairs across 8 NeuronCores.

Math (per (b,h) pair):
    scores = softmax(Q K^T / sqrt(64) + bias)   # bias broadcast over (b,h)
    out    = scores @ V

Device-side layout: computed TRANSPOSED (scoresT[k, q]) so the S x S scores
matrix never needs an on-chip transpose. The softmax normalization and the
final [d, q] -> [q, d] transpose happen HOST-side: the kernel emits, per
q-tile, outT[d, q] (rows 0..63) and sums[q] (row 64, via a ones-column
appended to V), unnormalized.

Engine plan (the exp of S^2 elements is the fundamental bottleneck):
  - PE: every matmul runs in 64-row tiling mode (tile_size (64,128)) so the
    engine never pays a tiling-mode drain:
      MM1: scoresT k-tile pairs as dual row-tiles (T0 = partitions 0-63,
           T8 = 64-127; Q,K duplicated into both partition halves host-side)
           -> 2 k-tiles per 512-cycle stream.
      MM2: each k-tile's 128-deep contraction split into two 64-row halves
           accumulating into separate PSUM banks o_a / o_b (concurrent
           duals); o = o_a + o_b folded into the qt drain.
  - bias: exp(s+b) = exp(s)*exp(b). Per k-tile group one of:
      "mult":  ACT exp (PSUM f32 -> SBUF bf16), then multiply by
               host-precomputed exp(biasT) (bf16) on DVE or GPSIMD.
      "sch":   DVE-only 2^x via int32 Schraudolph: one scalar_tensor_tensor
               i32 = rint(s*C + bprep[k,q]), C = 2^23*log2e, bprep folds the
               bias, the 127-exponent offset and a mean-centering shift;
               MM2 reads the int32 tile bitcast to float32r (1 cyc/row at
               N=512). Ripple of (1+f)/2^f costs ~6e-3 rel err at 5/16
               of k-tiles (measured vs reference: ~1.05e-2 total).
"""

import os
import sys

import numpy as np

for _p in ("/opt/trn_rl_repo",):
    if _p not in sys.path and os.path.isdir(_p):
        sys.path.insert(0, _p)

B, H, S, D = 2, 16, 2048, 64
N_CORES = 8
PAIRS = B * H                     # 32
PPC = PAIRS // N_CORES            # 4 pairs per core
SCALE = 1.0 / 8.0                 # 1/sqrt(64)

KT = S // 128                     # 16 k-tiles of 128
QTILE = 512
QT = S // QTILE                   # 4 q-tiles

LOG2E = 1.4426950408889634
# int16 Schraudolph: i16 = rint(s*C16 + bprep) bit-patterns a bf16 (1-8-7).
# The precompute tensor is f32 (DVE sums in f32 internally), so the large
# 127*128 exponent offset costs no precision; only the final int16 convert
# quantizes (2^-8 relative, same as the bf16 probs of the mult path).
C16 = float(np.float32(LOG2E * 128.0))
# exponent offset with mean-centering shift (E[(1+f)/2^f] = 1.0418)
SCH_OFF = float(np.float32((127.0 - 0.05915) * 128.0))

# per-qt k-tile groups: (kt0, n_kt, path). Paths: "mult_dve" / "mult_gps"
# (ACT exp + multiply on that engine) or "sch" (DVE Schraudolph).
# The two gps groups are spread apart (their 2.6-3us multiplies queue
# behind each other on the GPSIMD engine otherwise).
GROUPS = [
    (0, 3, "mult_gps"),
    (3, 3, "mult_dve"),
    (6, 3, "sch"),
    (9, 3, "sch"),
    (12, 2, "mult_gps"),
    (14, 2, "mult_dve"),
]
_env_paths = os.environ.get("KPATHS")
if _env_paths:
    _p = _env_paths.split(",")
    GROUPS = [(g[0], g[1], _p[i]) for i, g in enumerate(GROUPS)]

MULT_KTS = [g[0] + j for g in GROUPS if g[2].startswith("mult") for j in range(g[1])]
SCH_KTS = [g[0] + j for g in GROUPS if g[2] == "sch" for j in range(g[1])]
N_MULT = len(MULT_KTS)
N_SCH = len(SCH_KTS)
# group -> slot offset within its bias-class tensor
_moff, _soff, MULT_OFF, SCH_OFF_IDX = 0, 0, {}, {}
for gi, (kt0, n, path) in enumerate(GROUPS):
    if path == "sch":
        SCH_OFF_IDX[gi] = _soff
        _soff += n
    else:
        MULT_OFF[gi] = _moff
        _moff += n

LAG = int(os.environ.get("LAG", "5"))
PP_BUFS = int(os.environ.get("PP_BUFS", "7"))
P32_BUFS = int(os.environ.get("P32_BUFS", "4"))
SC_BUFS = int(os.environ.get("SC_BUFS", "2"))

_CACHE = {}


def _build_nc():
    import concourse.mybir as mybir
    import concourse.tile as tile
    from concourse import bacc

    f32 = mybir.dt.float32
    bf16 = mybir.dt.bfloat16
    i16 = mybir.dt.int16
    nc = bacc.Bacc(None)

    qT2 = nc.declare_dram_parameter("qT2", [PPC, 128, S], bf16, isOutput=False)
    kT2 = nc.declare_dram_parameter("kT2", [PPC, 128, S], bf16, isOutput=False)
    v1 = nc.declare_dram_parameter("v1", [PPC, 128, KT, D + 1], bf16, isOutput=False)
    ebT_d = nc.declare_dram_parameter("ebT", [128, max(N_MULT, 1), S], bf16, isOutput=False)
    bp_d = nc.declare_dram_parameter("bp32", [128, max(N_SCH, 1), S], f32, isOutput=False)
    out = nc.declare_dram_parameter("out", [PPC, 2, D + 1, S], f32, isOutput=True)

    mult = mybir.AluOpType.mult
    add = mybir.AluOpType.add
    Exp = mybir.ActivationFunctionType.Exp

    with tile.TileContext(nc) as tc:
        with (
            tc.tile_pool(name="bias", bufs=1) as bias_pool,
            tc.tile_pool(name="qk", bufs=2) as qk_pool,
            tc.tile_pool(name="vv", bufs=2) as v_pool,
            tc.tile_pool(name="probP", bufs=PP_BUFS) as pP_pool,
            tc.tile_pool(name="prob16", bufs=P32_BUFS) as p16_pool,
            tc.tile_pool(name="osb", bufs=2) as osb_pool,
            tc.tile_pool(name="sc", bufs=SC_BUFS, space="PSUM") as sc_pool,
            tc.tile_pool(name="oab", bufs=1, space="PSUM") as oab_pool,
        ):
            def load_pair(p):
                q_sb = qk_pool.tile([128, S], bf16, tag="q")
                nc.sync.dma_start(q_sb[:], qT2[p])
                k_sb = qk_pool.tile([128, S], bf16, tag="k")
                nc.sync.dma_start(k_sb[:], kT2[p])
                v_sb = v_pool.tile([128, KT, D + 1], bf16)
                nc.sync.dma_start(v_sb[:], v1[p])
                return q_sb, k_sb, v_sb

            loaded = {0: load_pair(0)}

            # bias-derived tensors resident in SBUF; DMA'd q-column-major so
            # the qt=0 slices land first.
            ebT_sb = bias_pool.tile([128, max(N_MULT, 1), S], bf16)
            bp_sb = bias_pool.tile([128, max(N_SCH, 1), S], f32)
            for qc in range(QT):
                qsl = slice(qc * QTILE, (qc + 1) * QTILE)
                if N_MULT:
                    nc.sync.dma_start(ebT_sb[:, :N_MULT, qsl], ebT_d[:, :N_MULT, qsl])
                if N_SCH:
                    nc.sync.dma_start(bp_sb[:, :N_SCH, qsl], bp_d[:, :N_SCH, qsl])

            def produce(p, qt, gi):
                q_sb, k_sb, _ = loaded[p]
                kt0, n, path = GROUPS[gi]
                qsl = slice(qt * QTILE, (qt + 1) * QTILE)
                s_ps = sc_pool.tile([128, 3, QTILE], f32, tag="sc")
                # MM1: dual row-tiled pairs (T0 = partitions 0-63 computes
                # k-tile kt0+2j, T8 = 64-127 computes kt0+2j+1), then a
                # lone T0 matmul for an odd trailing k-tile.
                for j in range(0, n - 1, 2):
                    ka, kb = kt0 + j, kt0 + j + 1
                    nc.tensor.matmul(
                        s_ps[:, j, :],
                        k_sb[0:64, ka * 128 : (ka + 1) * 128],
                        q_sb[0:64, qsl],
                        start=True, stop=True,
                    )
                    nc.tensor.matmul(
                        s_ps[:, j + 1, :],
                        k_sb[64:128, kb * 128 : (kb + 1) * 128],
                        q_sb[64:128, qsl],
                        start=True, stop=True,
                    )
                if n % 2:
                    kc = kt0 + n - 1
                    nc.tensor.matmul(
                        s_ps[:, n - 1, :],
                        k_sb[0:64, kc * 128 : (kc + 1) * 128],
                        q_sb[0:64, qsl],
                        start=True, stop=True,
                    )
                # Postproc of 3-kt groups is split [slots 0-1] + [slot 2]:
                # the next MM1 dual into this PSUM tile overwrites slots 0-1
                # first, and subtile WAR tracking lets it start as soon as the
                # first (smaller) read completes — shortens the 2-buffer
                # score-bank rotation that otherwise paces the whole kernel.
                splits = [(0, 2), (2, n)] if n == 3 else [(0, n)]
                if path == "sch":
                    soff = SCH_OFF_IDX[gi]
                    p16 = p16_pool.tile([128, 3, QTILE], i16, tag="p16")
                    for lo, hi in splits:
                        nc.vector.scalar_tensor_tensor(
                            p16[:, lo:hi, :],
                            s_ps[:, lo:hi, :],
                            C16,
                            bp_sb[:, soff + lo : soff + hi, qsl],
                            op0=mult,
                            op1=add,
                        )
                    return (kt0, n, p16, True)
                moff = MULT_OFF[gi]
                p_sb = pP_pool.tile([128, 3, QTILE], bf16, tag="pP")
                for lo, hi in splits:
                    nc.scalar.activation(
                        p_sb[:, lo:hi, :], s_ps[:, lo:hi, :], Exp
                    )
                eng = nc.gpsimd if path == "mult_gps" else nc.vector
                eng.tensor_tensor(
                    p_sb[:, :n, :],
                    p_sb[:, :n, :],
                    ebT_sb[:, moff : moff + n, qsl],
                    mult,
                )
                return (kt0, n, p_sb, False)

            def consume(p, qt, chunk, st):
                _, _, v_sb = loaded[p]
                kt0, n, p_t, is16 = chunk
                o_a, o_b = st["o_a"], st["o_b"]
                base = st["consumed"]
                for j in range(n):
                    kt = kt0 + j
                    rhs = p_t[:, j, :]
                    if is16:
                        rhs = rhs.bitcast(bf16)
                    # groups are consumed out of kt order (per-path lags), so
                    # the PSUM accumulation flags key on consume position
                    start = base + j == 0
                    stop = base + j == KT - 1
                    nc.tensor.matmul(
                        o_a[:],
                        v_sb[0:64, kt, :],
                        rhs[0:64, :],
                        start=start, stop=stop,
                    )
                    nc.tensor.matmul(
                        o_b[:],
                        v_sb[64:128, kt, :],
                        rhs[64:128, :],
                        start=start, stop=stop,
                    )
                st["consumed"] += n

            def epilogue(p, qt, st):
                # Drain o_a via ACT and o_b via DVE as two INDEPENDENT
                # copies (DVE can read only one PSUM operand per op, and a
                # staged add would chain the two engines); the host adds the
                # halves together with the softmax normalization.
                qsl = slice(qt * QTILE, (qt + 1) * QTILE)
                t_sb = osb_pool.tile([D + 1, QTILE], f32, tag="ta")
                nc.scalar.copy(t_sb[:], st["o_a"][:])
                nc.sync.dma_start(out[p, 0, :, qsl], t_sb[:])
                o_sb = osb_pool.tile([D + 1, QTILE], f32, tag="osb")
                nc.vector.tensor_scalar_mul(o_sb[:], st["o_b"][:], 1.0)
                nc.sync.dma_start(out[p, 1, :, qsl], o_sb[:])

            stream = []
            for p in range(PPC):
                for qt in range(QT):
                    for gi in range(len(GROUPS)):
                        stream.append((p, qt, gi))

            state = {}
            pending = []

            def consume_one():
                pp, pq, pc = pending.pop(0)
                st = state[(pp, pq)]
                consume(pp, pq, pc, st)
                if st["consumed"] == KT:
                    epilogue(pp, pq, state.pop((pp, pq)))

            for p, qt, gi in stream:
                if p not in loaded:
                    loaded[p] = load_pair(p)
                if qt == QT - 1 and p + 1 < PPC and p + 1 not in loaded:
                    loaded[p + 1] = load_pair(p + 1)
                for old in [k for k in loaded if k < p - 1]:
                    del loaded[old]
                if (p, qt) not in state:
                    o_a = oab_pool.tile([D + 1, QTILE], f32, name="oa", tag="oa")
                    o_b = oab_pool.tile([D + 1, QTILE], f32, name="ob", tag="ob")
                    state[(p, qt)] = {"o_a": o_a, "o_b": o_b, "consumed": 0}
                # consume the oldest ready chunk BEFORE producing: its probs
                # are older, so the PE queue head rarely stalls on the wait.
                if len(pending) >= LAG:
                    consume_one()
                chunk = produce(p, qt, gi)
                pending.append((p, qt, chunk))
            while pending:
                consume_one()

    return nc


def _get_nc():
    if "nc" not in _CACHE:
        nc = _build_nc()
        nc.finalize()
        _CACHE["nc"] = nc
    return _CACHE["nc"]


def _make_in_maps(mat1, mat2, mat3, bias):
    import ml_dtypes

    bf16 = ml_dtypes.bfloat16
    q = (np.asarray(mat1, dtype=np.float32).reshape(PAIRS, S, D) * SCALE)
    k = np.asarray(mat2, dtype=np.float32).reshape(PAIRS, S, D)
    v = np.asarray(mat3, dtype=np.float32).reshape(PAIRS, S, D)

    qT = q.transpose(0, 2, 1).astype(bf16)          # [P, 64, S]
    kT = k.transpose(0, 2, 1).astype(bf16)
    qT2 = np.concatenate([qT, qT], axis=1)          # [P, 128, S]
    kT2 = np.concatenate([kT, kT], axis=1)
    qT2 = np.ascontiguousarray(qT2)
    kT2 = np.ascontiguousarray(kT2)

    v1f = np.concatenate([v, np.ones((PAIRS, S, 1), np.float32)], axis=2)
    # [P, S, 65] -> [P, 128, KT, 65]: partition = k % 128, slot = k // 128
    v1t = v1f.reshape(PAIRS, KT, 128, D + 1).transpose(0, 2, 1, 3)
    v1 = np.ascontiguousarray(v1t.astype(bf16))

    bT = np.asarray(bias, dtype=np.float32).reshape(S, S).T  # [k, q]
    # [k, q] -> [128, KT, S] (partition = k % 128)
    bT_t = np.ascontiguousarray(bT.reshape(KT, 128, S).transpose(1, 0, 2))
    if N_MULT:
        ebT = np.ascontiguousarray(np.exp(bT_t[:, MULT_KTS, :]).astype(bf16))
    else:
        ebT = np.zeros((128, 1, S), bf16)
    if N_SCH:
        bp32 = np.ascontiguousarray(
            (bT_t[:, SCH_KTS, :] * np.float32(C16) + np.float32(SCH_OFF)).astype(
                np.float32
            )
        )
    else:
        bp32 = np.zeros((128, 1, S), np.float32)

    in_maps = []
    for c in range(N_CORES):
        sl = slice(c * PPC, (c + 1) * PPC)
        in_maps.append(
            {
                "qT2": qT2[sl],
                "kT2": kT2[sl],
                "v1": v1[sl],
                "ebT": ebT,
                "bp32": bp32,
            }
        )
    return in_maps


def kernel(mat1, mat2, mat3, bias):
    from concourse.bass_utils import run_bass_kernel_spmd

    in_maps = _make_in_maps(mat1, mat2, mat3, bias)
    nc = _get_nc()
    _CACHE["in_maps"] = in_maps
    res = run_bass_kernel_spmd(nc, in_maps, list(range(N_CORES)))
    outs = [res.results[c]["out"] for c in range(N_CORES)]
    o2 = np.concatenate(outs, axis=0)                # [PAIRS, 2, 65, S] f32
    o = o2[:, 0] + o2[:, 1]                          # [PAIRS, 65, S]
    full = (o[:, :D, :] / o[:, D : D + 1, :]).transpose(0, 2, 1)
    return np.ascontiguousarray(full.reshape(B, H, S, D).astype(np.float32))


# revision 43
# speedup vs baseline: 1.2576x; 1.1732x over previous
"""MHA forward kernel for Trainium2 (Bass/Tile), sharded over (batch, head)
pairs across 8 NeuronCores.

Math (per (b,h) pair):
    scores = softmax(Q K^T / sqrt(64) + bias)   # bias broadcast over (b,h)
    out    = scores @ V

Device-side layout: computed TRANSPOSED (scoresT[k, q]) so the S x S scores
matrix never needs an on-chip transpose. The softmax normalization and the
final [d, q] -> [q, d] transpose happen HOST-side: the kernel emits, per
q-tile, outT[d, q] (rows 0..63) and sums[q] (row 64, via a ones-column
appended to V), unnormalized.

Engine plan (the exp of S^2 elements is the fundamental bottleneck):
  - PE: every matmul runs in 64-row tiling mode (tile_size (64,128)) so the
    engine never pays a tiling-mode drain:
      MM1: scoresT k-tile pairs as dual row-tiles (T0 = partitions 0-63,
           T8 = 64-127; Q,K duplicated into both partition halves host-side)
           -> 2 k-tiles per 512-cycle stream.
      MM2: each k-tile's 128-deep contraction split into two 64-row halves
           accumulating into separate PSUM banks o_a / o_b (concurrent
           duals); o = o_a + o_b folded into the qt drain.
  - bias: exp(s+b) = exp(s)*exp(b). Per k-tile group one of:
      "mult":  ACT exp (PSUM f32 -> SBUF bf16), then multiply by
               host-precomputed exp(biasT) (bf16) on DVE or GPSIMD.
      "sch":   DVE-only 2^x via int32 Schraudolph: one scalar_tensor_tensor
               i32 = rint(s*C + bprep[k,q]), C = 2^23*log2e, bprep folds the
               bias, the 127-exponent offset and a mean-centering shift;
               MM2 reads the int32 tile bitcast to float32r (1 cyc/row at
               N=512). Ripple of (1+f)/2^f costs ~6e-3 rel err at 5/16
               of k-tiles (measured vs reference: ~1.05e-2 total).
"""

import os
import sys

import numpy as np

for _p in ("/opt/trn_rl_repo",):
    if _p not in sys.path and os.path.isdir(_p):
        sys.path.insert(0, _p)

B, H, S, D = 2, 16, 2048, 64
N_CORES = 8
PAIRS = B * H                     # 32
PPC = PAIRS // N_CORES            # 4 pairs per core
SCALE = 1.0 / 8.0                 # 1/sqrt(64)

KT = S // 128                     # 16 k-tiles of 128
QTILE = 512
QT = S // QTILE                   # 4 q-tiles

LOG2E = 1.4426950408889634
# int16 Schraudolph: i16 = rint(s*C16 + bprep) bit-patterns a bf16 (1-8-7).
# The precompute tensor is f32 (DVE sums in f32 internally), so the large
# 127*128 exponent offset costs no precision; only the final int16 convert
# quantizes (2^-8 relative, same as the bf16 probs of the mult path).
C16 = float(np.float32(LOG2E * 128.0))
# exponent offset with mean-centering shift (E[(1+f)/2^f] = 1.0418)
SCH_OFF = float(np.float32((127.0 - 0.05915) * 128.0))

# per-qt k-tile groups: (kt0, n_kt, path). Paths: "mult_dve" / "mult_gps"
# (ACT exp + multiply on that engine) or "sch" (DVE Schraudolph).
# Group paths and their consume lags (in stream steps). The lags match each
# path's probs-chain latency (gps ~4.5us, dve-mult ~2.5us, sch ~2us) while
# keeping the LAST consume of a qt early: the o_a/o_b PSUM banks are
# single-buffered, so the qt drain must clear the DVE queue before the next
# qt's first MM2 (two steps of slack here).
GROUPS = [
    (0, 3, "mult_gps"),
    (3, 3, "mult_dve"),
    (6, 3, "mult_gps"),
    (9, 3, "sch"),
    (12, 2, "mult_dve"),
    (14, 2, "sch"),
]
LAGS = [4, 3, 5, 2, 3, 2]
if os.environ.get("KLAGS"):
    LAGS = [int(x) for x in os.environ["KLAGS"].split(",")]
_env_paths = os.environ.get("KPATHS")
if _env_paths:
    _p = _env_paths.split(",")
    GROUPS = [(g[0], g[1], _p[i]) for i, g in enumerate(GROUPS)]

MULT_KTS = [g[0] + j for g in GROUPS if g[2].startswith("mult") for j in range(g[1])]
SCH_KTS = [g[0] + j for g in GROUPS if g[2] == "sch" for j in range(g[1])]
N_MULT = len(MULT_KTS)
N_SCH = len(SCH_KTS)
# group -> slot offset within its bias-class tensor
_moff, _soff, MULT_OFF, SCH_OFF_IDX = 0, 0, {}, {}
for gi, (kt0, n, path) in enumerate(GROUPS):
    if path == "sch":
        SCH_OFF_IDX[gi] = _soff
        _soff += n
    else:
        MULT_OFF[gi] = _moff
        _moff += n

LAG = int(os.environ.get("LAG", "5"))
PP_BUFS = int(os.environ.get("PP_BUFS", "7"))
P32_BUFS = int(os.environ.get("P32_BUFS", "4"))
SC_BUFS = int(os.environ.get("SC_BUFS", "2"))

_CACHE = {}


def _build_nc():
    import concourse.mybir as mybir
    import concourse.tile as tile
    from concourse import bacc

    f32 = mybir.dt.float32
    bf16 = mybir.dt.bfloat16
    i16 = mybir.dt.int16
    nc = bacc.Bacc(None)

    qT2 = nc.declare_dram_parameter("qT2", [PPC, 128, S], bf16, isOutput=False)
    kT2 = nc.declare_dram_parameter("kT2", [PPC, 128, S], bf16, isOutput=False)
    v1 = nc.declare_dram_parameter("v1", [PPC, 128, KT, D + 1], bf16, isOutput=False)
    ebT_d = nc.declare_dram_parameter("ebT", [128, max(N_MULT, 1), S], bf16, isOutput=False)
    bp_d = nc.declare_dram_parameter("bp32", [128, max(N_SCH, 1), S], f32, isOutput=False)
    out = nc.declare_dram_parameter("out", [PPC, 2, D + 1, S], f32, isOutput=True)

    mult = mybir.AluOpType.mult
    add = mybir.AluOpType.add
    Exp = mybir.ActivationFunctionType.Exp

    with tile.TileContext(nc) as tc:
        with (
            tc.tile_pool(name="bias", bufs=1) as bias_pool,
            tc.tile_pool(name="qk", bufs=2) as qk_pool,
            tc.tile_pool(name="vv", bufs=2) as v_pool,
            tc.tile_pool(name="probP", bufs=PP_BUFS) as pP_pool,
            tc.tile_pool(name="prob16", bufs=P32_BUFS) as p16_pool,
            tc.tile_pool(name="osb", bufs=2) as osb_pool,
            tc.tile_pool(name="sc", bufs=SC_BUFS, space="PSUM") as sc_pool,
            tc.tile_pool(name="oab", bufs=1, space="PSUM") as oab_pool,
        ):
            def load_pair(p):
                q_sb = qk_pool.tile([128, S], bf16, tag="q")
                nc.sync.dma_start(q_sb[:], qT2[p])
                k_sb = qk_pool.tile([128, S], bf16, tag="k")
                nc.sync.dma_start(k_sb[:], kT2[p])
                v_sb = v_pool.tile([128, KT, D + 1], bf16)
                nc.sync.dma_start(v_sb[:], v1[p])
                return q_sb, k_sb, v_sb

            loaded = {0: load_pair(0)}

            # bias-derived tensors resident in SBUF; DMA'd q-column-major so
            # the qt=0 slices land first.
            ebT_sb = bias_pool.tile([128, max(N_MULT, 1), S], bf16)
            bp_sb = bias_pool.tile([128, max(N_SCH, 1), S], f32)
            for qc in range(QT):
                qsl = slice(qc * QTILE, (qc + 1) * QTILE)
                if N_MULT:
                    nc.sync.dma_start(ebT_sb[:, :N_MULT, qsl], ebT_d[:, :N_MULT, qsl])
                if N_SCH:
                    nc.sync.dma_start(bp_sb[:, :N_SCH, qsl], bp_d[:, :N_SCH, qsl])

            def produce(p, qt, gi):
                q_sb, k_sb, _ = loaded[p]
                kt0, n, path = GROUPS[gi]
                qsl = slice(qt * QTILE, (qt + 1) * QTILE)
                s_ps = sc_pool.tile([128, 3, QTILE], f32, tag="sc")
                # MM1: dual row-tiled pairs (T0 = partitions 0-63 computes
                # k-tile kt0+2j, T8 = 64-127 computes kt0+2j+1), then a
                # lone T0 matmul for an odd trailing k-tile.
                for j in range(0, n - 1, 2):
                    ka, kb = kt0 + j, kt0 + j + 1
                    nc.tensor.matmul(
                        s_ps[:, j, :],
                        k_sb[0:64, ka * 128 : (ka + 1) * 128],
                        q_sb[0:64, qsl],
                        start=True, stop=True,
                    )
                    nc.tensor.matmul(
                        s_ps[:, j + 1, :],
                        k_sb[64:128, kb * 128 : (kb + 1) * 128],
                        q_sb[64:128, qsl],
                        start=True, stop=True,
                    )
                if n % 2:
                    kc = kt0 + n - 1
                    nc.tensor.matmul(
                        s_ps[:, n - 1, :],
                        k_sb[0:64, kc * 128 : (kc + 1) * 128],
                        q_sb[0:64, qsl],
                        start=True, stop=True,
                    )
                # Postproc of 3-kt groups is split [slots 0-1] + [slot 2]:
                # the next MM1 dual into this PSUM tile overwrites slots 0-1
                # first, and subtile WAR tracking lets it start as soon as the
                # first (smaller) read completes — shortens the 2-buffer
                # score-bank rotation that otherwise paces the whole kernel.
                splits = [(0, 2), (2, n)] if n == 3 else [(0, n)]
                if path == "sch":
                    soff = SCH_OFF_IDX[gi]
                    p16 = p16_pool.tile([128, 3, QTILE], i16, tag="p16")
                    for lo, hi in splits:
                        nc.vector.scalar_tensor_tensor(
                            p16[:, lo:hi, :],
                            s_ps[:, lo:hi, :],
                            C16,
                            bp_sb[:, soff + lo : soff + hi, qsl],
                            op0=mult,
                            op1=add,
                        )
                    return (kt0, n, p16, True)
                moff = MULT_OFF[gi]
                p_sb = pP_pool.tile([128, 3, QTILE], bf16, tag="pP")
                for lo, hi in splits:
                    nc.scalar.activation(
                        p_sb[:, lo:hi, :], s_ps[:, lo:hi, :], Exp
                    )
                eng = nc.gpsimd if path == "mult_gps" else nc.vector
                eng.tensor_tensor(
                    p_sb[:, :n, :],
                    p_sb[:, :n, :],
                    ebT_sb[:, moff : moff + n, qsl],
                    mult,
                )
                return (kt0, n, p_sb, False)

            def consume(p, qt, chunk, st):
                _, _, v_sb = loaded[p]
                kt0, n, p_t, is16 = chunk
                o_a, o_b = st["o_a"], st["o_b"]
                base = st["consumed"]
                for j in range(n):
                    kt = kt0 + j
                    rhs = p_t[:, j, :]
                    if is16:
                        rhs = rhs.bitcast(bf16)
                    # groups are consumed out of kt order (per-path lags), so
                    # the PSUM accumulation flags key on consume position
                    start = base + j == 0
                    stop = base + j == KT - 1
                    nc.tensor.matmul(
                        o_a[:],
                        v_sb[0:64, kt, :],
                        rhs[0:64, :],
                        start=start, stop=stop,
                    )
                    nc.tensor.matmul(
                        o_b[:],
                        v_sb[64:128, kt, :],
                        rhs[64:128, :],
                        start=start, stop=stop,
                    )
                st["consumed"] += n

            def epilogue(p, qt, st):
                # Drain o_a / o_b as two independent DVE copies (DVE can read
                # only one PSUM operand per op); the host adds the halves
                # together with the softmax normalization.
                qsl = slice(qt * QTILE, (qt + 1) * QTILE)
                t_sb = osb_pool.tile([D + 1, QTILE], f32, tag="ta")
                nc.vector.tensor_scalar_mul(t_sb[:], st["o_a"][:], 1.0)
                nc.sync.dma_start(out[p, 0, :, qsl], t_sb[:])
                o_sb = osb_pool.tile([D + 1, QTILE], f32, tag="osb")
                nc.vector.tensor_scalar_mul(o_sb[:], st["o_b"][:], 1.0)
                nc.sync.dma_start(out[p, 1, :, qsl], o_sb[:])

            stream = []
            for p in range(PPC):
                for qt in range(QT):
                    for gi in range(len(GROUPS)):
                        stream.append((p, qt, gi))

            state = {}
            pending = []  # (due_step, seq, p, qt, chunk)

            def drain_due(step):
                for c in sorted(x for x in pending if x[0] <= step):
                    pending.remove(c)
                    _, _, pp, pq, pc = c
                    st = state[(pp, pq)]
                    consume(pp, pq, pc, st)
                    if st["consumed"] == KT:
                        epilogue(pp, pq, state.pop((pp, pq)))

            for step, (p, qt, gi) in enumerate(stream):
                if p not in loaded:
                    loaded[p] = load_pair(p)
                if qt == QT - 1 and p + 1 < PPC and p + 1 not in loaded:
                    loaded[p + 1] = load_pair(p + 1)
                for old in [k for k in loaded if k < p - 1]:
                    del loaded[old]
                if (p, qt) not in state:
                    o_a = oab_pool.tile([D + 1, QTILE], f32, name="oa", tag="oa")
                    o_b = oab_pool.tile([D + 1, QTILE], f32, name="ob", tag="ob")
                    state[(p, qt)] = {"o_a": o_a, "o_b": o_b, "consumed": 0}
                # consume due chunks BEFORE producing: their probs are old,
                # so the PE queue head rarely stalls on the wait.
                drain_due(step)
                chunk = produce(p, qt, gi)
                pending.append((step + LAGS[gi], step, p, qt, chunk))
            step = len(stream)
            while pending:
                drain_due(step)
                step += 1

    return nc


def _get_nc():
    if "nc" not in _CACHE:
        nc = _build_nc()
        nc.finalize()
        _CACHE["nc"] = nc
    return _CACHE["nc"]


def _make_in_maps(mat1, mat2, mat3, bias):
    import ml_dtypes

    bf16 = ml_dtypes.bfloat16
    q = (np.asarray(mat1, dtype=np.float32).reshape(PAIRS, S, D) * SCALE)
    k = np.asarray(mat2, dtype=np.float32).reshape(PAIRS, S, D)
    v = np.asarray(mat3, dtype=np.float32).reshape(PAIRS, S, D)

    qT = q.transpose(0, 2, 1).astype(bf16)          # [P, 64, S]
    kT = k.transpose(0, 2, 1).astype(bf16)
    qT2 = np.concatenate([qT, qT], axis=1)          # [P, 128, S]
    kT2 = np.concatenate([kT, kT], axis=1)
    qT2 = np.ascontiguousarray(qT2)
    kT2 = np.ascontiguousarray(kT2)

    v1f = np.concatenate([v, np.ones((PAIRS, S, 1), np.float32)], axis=2)
    # [P, S, 65] -> [P, 128, KT, 65]: partition = k % 128, slot = k // 128
    v1t = v1f.reshape(PAIRS, KT, 128, D + 1).transpose(0, 2, 1, 3)
    v1 = np.ascontiguousarray(v1t.astype(bf16))

    bT = np.asarray(bias, dtype=np.float32).reshape(S, S).T  # [k, q]
    # [k, q] -> [128, KT, S] (partition = k % 128)
    bT_t = np.ascontiguousarray(bT.reshape(KT, 128, S).transpose(1, 0, 2))
    if N_MULT:
        ebT = np.ascontiguousarray(np.exp(bT_t[:, MULT_KTS, :]).astype(bf16))
    else:
        ebT = np.zeros((128, 1, S), bf16)
    if N_SCH:
        bp32 = np.ascontiguousarray(
            (bT_t[:, SCH_KTS, :] * np.float32(C16) + np.float32(SCH_OFF)).astype(
                np.float32
            )
        )
    else:
        bp32 = np.zeros((128, 1, S), np.float32)

    in_maps = []
    for c in range(N_CORES):
        sl = slice(c * PPC, (c + 1) * PPC)
        in_maps.append(
            {
                "qT2": qT2[sl],
                "kT2": kT2[sl],
                "v1": v1[sl],
                "ebT": ebT,
                "bp32": bp32,
            }
        )
    return in_maps


def kernel(mat1, mat2, mat3, bias):
    from concourse.bass_utils import run_bass_kernel_spmd

    in_maps = _make_in_maps(mat1, mat2, mat3, bias)
    nc = _get_nc()
    _CACHE["in_maps"] = in_maps
    res = run_bass_kernel_spmd(nc, in_maps, list(range(N_CORES)))
    outs = [res.results[c]["out"] for c in range(N_CORES)]
    o2 = np.concatenate(outs, axis=0)                # [PAIRS, 2, 65, S] f32
    o = o2[:, 0] + o2[:, 1]                          # [PAIRS, 65, S]
    full = (o[:, :D, :] / o[:, D : D + 1, :]).transpose(0, 2, 1)
    return np.ascontiguousarray(full.reshape(B, H, S, D).astype(np.float32))
